# revision 1
# baseline (speedup 1.0000x reference)
"""Trainium2 Bass kernel for the e3nn-style GNN convolution layer.

kernel(**inputs) takes FULL (unsharded) numpy inputs and returns the FULL
[N, 160] float32 output.  Internally shards edges across 8 NeuronCores by
destination-node range, runs one SPMD Bass program, and reassembles on host.

Design:
  host prep   - fold all scalar normalizations into weights; x~ = node_input *
                node_attr; transpose node features (1o block c-major); sort
                edges by (dst-window, src-table-half), pad each half-group to
                a fixed number of 128-edge tiles (padding has edge_attr = 0 so
                its messages vanish).
  phase A     - per 128-node window: one fused [s | y] float32r matmul
                (self-connection s kept in SBUF, lin1 output y written bf16);
                AllGather replicates the y table across the 8 cores.
  edge phase  - per window: 2 dma_gather calls fetch y[src] rows (512B each)
                from the two table halves; per 128-edge tile: FC net (2 bf16
                matmuls + Silu), message build on DVE/ACT, and a one-hot
                selection matmul accumulating z in PSUM.
  node phase  - per window: transpose z (PE), lin2 (4 bf16 matmuls), add s,
                write the output slice.
"""

import math
from dataclasses import dataclass

import numpy as np
import ml_dtypes

import concourse.bacc as bacc
import concourse.bass as bass
import concourse.mybir as mybir
import concourse.tile as tile
from concourse.bass_utils import run_bass_kernel_spmd
from concourse.masks import make_identity

USE_ALLGATHER = False
BF16 = ml_dtypes.bfloat16
F32 = np.float32

MUL0 = 64
MUL1 = 32
FC_IN = 16
FC_H = 64
WN = 2 * MUL0 + 2 * MUL1  # 192 per-edge tp weights
D_IN = MUL0 + 3 * MUL1    # 160
DE = 256                  # padded y-table row elems (512 B in bf16)
D_MID = 4 * (MUL0 + MUL1) # 384 = [z0 (96) | z1_c0 | z1_c1 | z1_c2]
NUM_NEIGH = 10.0
C_S = math.sin(math.pi / 8.0)
C_X = math.cos(math.pi / 8.0)
P = 128


@dataclass(frozen=True)
class Cfg:
    n: int
    n_cores: int
    npc: int          # nodes per core
    wpc: int          # 128-node windows per core
    npad: int         # wpc * 128
    th: int           # tiles per (window, table-half)
    use_allgather: bool = True
    repeat: int = 1
    null: bool = False
    # ablation flags (timing experiments only; break correctness)
    ab_no_gather: bool = False
    ab_no_fc: bool = False
    ab_no_msg: bool = False
    ab_no_sel: bool = False
    ab_no_s4: bool = False
    ab_no_node: bool = False


def _to_cmajor(x_uc):
    s = x_uc.shape[:-1]
    return x_uc.reshape(*s, MUL1, 3).swapaxes(-1, -2).reshape(*s, 96)


def _from_cmajor(x_cu):
    s = x_cu.shape[:-1]
    return x_cu.reshape(*s, 3, MUL1).swapaxes(-1, -2).reshape(*s, 96)


# ---------------------------------------------------------------- host prep

def _prep(inputs, n_cores=8):
    node_input = np.asarray(inputs["node_input"], F32)
    node_attr = np.asarray(inputs["node_attr"], F32)
    edge_src = np.asarray(inputs["edge_src"]).astype(np.int64)
    edge_dst = np.asarray(inputs["edge_dst"]).astype(np.int64)
    edge_attr = np.asarray(inputs["edge_attr"], F32)
    ele = np.asarray(inputs["edge_length_embedded"], F32)

    n = node_input.shape[0]
    e = edge_src.shape[0]
    assert n % n_cores == 0
    npc = n // n_cores
    wpc = (npc + P - 1) // P
    npad = wpc * P
    ntab = n_cores * npad
    half = ntab // 2
    assert half <= 32767 and half % npad == 0

    inv0 = 1.0 / math.sqrt(MUL0)
    inv1 = 1.0 / math.sqrt(MUL1)
    invm = 1.0 / math.sqrt(MUL0 + MUL1)
    invnb = 1.0 / math.sqrt(NUM_NEIGH)

    x = node_input * node_attr
    xT = np.concatenate([x[:, :MUL0], _to_cmajor(x[:, MUL0:])], axis=1).T  # [160,n]
    xT = np.ascontiguousarray(xT, F32)

    W_sc0 = np.asarray(inputs["W_sc0"], F32) * (inv0 * C_S)
    W_sc1 = np.asarray(inputs["W_sc1"], F32) * (inv1 * C_S)
    W_l1_0 = np.asarray(inputs["W_l1_0"], F32) * inv0
    W_l1_1 = np.asarray(inputs["W_l1_1"], F32) * inv1
    fc_W1 = np.asarray(inputs["fc_W1"], F32) * (1.0 / math.sqrt(FC_IN))
    fc_W2 = np.asarray(inputs["fc_W2"], F32) * (1.0 / math.sqrt(FC_H))
    obase = invm * C_X * invnb
    W_l2_0 = np.asarray(inputs["W_l2_0"], F32) * obase
    W_l2_0 = W_l2_0.copy()
    W_l2_0[MUL0:, :] *= 1.0 / math.sqrt(3.0)
    W_l2_1 = np.asarray(inputs["W_l2_1"], F32) * obase

    def blockdiag(*ms):
        rows = sum(m.shape[0] for m in ms)
        cols = sum(m.shape[1] for m in ms)
        out = np.zeros((rows, cols), F32)
        r = c = 0
        for m in ms:
            out[r:r + m.shape[0], c:c + m.shape[1]] = m
            r += m.shape[0]
            c += m.shape[1]
        return out

    Wsc_big = blockdiag(W_sc0, W_sc1, W_sc1, W_sc1)
    Wl1_big = blockdiag(W_l1_0, W_l1_1, W_l1_1, W_l1_1)
    AB = np.ascontiguousarray(np.concatenate([Wsc_big, Wl1_big], axis=1), F32)

    # ---- edge sharding: (dst-window, src-half) groups
    core = edge_dst // npc
    local = edge_dst - core * npc
    win = local // P
    ldst = (local - win * P).astype(F32)
    src_remap = (edge_src // npc) * npad + (edge_src % npc)
    hbit = (src_remap >= half).astype(np.int64)
    g2 = (core * wpc + win) * 2 + hbit              # (window, half) group id
    order = np.argsort(g2, kind="stable")
    cnt2 = np.bincount(g2, minlength=n_cores * wpc * 2)
    th = max(1, int((cnt2.max() + P - 1) // P))     # tiles per half-group
    ni = th * P                                     # slots per half-group
    epw = 2 * ni                                    # edge slots per window
    tt = 2 * th                                     # tiles per window

    starts = np.zeros(n_cores * wpc * 2, np.int64)
    starts[1:] = np.cumsum(cnt2)[:-1]
    j_within = np.arange(e) - starts[g2[order]]
    dest = g2[order] * ni + j_within                # flat padded slot

    flat = n_cores * wpc * epw
    A_ = np.zeros((flat, 4), F32)
    A_[dest] = edge_attr[order]
    IDX = np.zeros(flat, np.int16)
    IDX[dest] = (src_remap[order] - hbit[order] * half).astype(np.int16)
    L_ = np.zeros(flat, F32)
    L_[dest] = ldst[order]
    E_ = np.zeros((flat, FC_IN), F32)
    E_[dest] = ele[order]

    E1_ = A_[:, 1:4]                                  # [flat, 3] (pre-transpose)
    A_ = A_.reshape(n_cores, wpc, tt, P, 4).transpose(0, 1, 3, 2, 4)
    attr_p = np.ascontiguousarray(A_.reshape(n_cores, wpc, P, tt * 4), F32)
    E8 = np.repeat(E1_[:, :, None], 8, axis=2)        # [flat, 3, 8]
    E8 = E8.reshape(n_cores, wpc, tt, P, 24).transpose(0, 1, 3, 2, 4)
    attr8_p = np.ascontiguousarray(E8.reshape(n_cores, wpc, P, tt * 24), BF16)
    ldst_p = np.ascontiguousarray(
        L_.reshape(n_cores, wpc, tt, P).transpose(0, 1, 3, 2), BF16)
    eleT_p = np.ascontiguousarray(
        E_.reshape(n_cores, wpc * tt * P, FC_IN).transpose(0, 2, 1), BF16)
    # idx wrapped for dma_gather: j -> (j%16, j//16), replicated over 8 groups
    ni16 = ni // 16
    IW = IDX.reshape(n_cores, wpc, 2, ni16, 16).swapaxes(3, 4)  # [c,w,h,16,ni16]
    idx_p = np.ascontiguousarray(
        np.broadcast_to(IW[:, :, :, None, :, :],
                        (n_cores, wpc, 2, 8, 16, ni16))
        .reshape(n_cores, wpc, 2, P, ni16))

    xT_pad = np.zeros((n_cores, D_IN, npad), F32)
    for k in range(n_cores):
        xT_pad[k, :, :npc] = xT[:, k * npc:(k + 1) * npc]
    xTbf_full = np.ascontiguousarray(
        xT_pad.transpose(1, 0, 2).reshape(D_IN, ntab), BF16)

    cfg = Cfg(n=n, n_cores=n_cores, npc=npc, wpc=wpc, npad=npad, th=th,
              use_allgather=USE_ALLGATHER)

    in_maps = []
    for k in range(n_cores):
        m = {
            "xTf": np.ascontiguousarray(xT_pad[k]),
            "AB_w": AB,
            "eleT": eleT_p[k],
            "attr_p": attr_p[k],
            "attr8_p": attr8_p[k],
            "idx_p": idx_p[k],
            "ldst_p": ldst_p[k],
            "fcW1": np.ascontiguousarray(fc_W1, BF16),
            "fcW2": np.ascontiguousarray(fc_W2, BF16),
            "Wl2_0c": np.ascontiguousarray(W_l2_0, BF16),
            "Wl2_1c": np.ascontiguousarray(W_l2_1, BF16),
        }
        if not cfg.use_allgather:
            m["xTbf"] = xTbf_full
            m["Wl1b"] = np.ascontiguousarray(Wl1_big, BF16)
        in_maps.append(m)
    return cfg, in_maps, node_attr


# ---------------------------------------------------------------- device program

_PROG_CACHE = {}


def _build(cfg: Cfg):
    if cfg in _PROG_CACHE:
        return _PROG_CACHE[cfg]

    th, wpc, npad = cfg.th, cfg.wpc, cfg.npad
    tt = 2 * th
    ni = th * P
    ni16 = ni // 16
    ep = wpc * tt * P
    ntab = cfg.n_cores * npad
    half = ntab // 2
    bf = mybir.dt.bfloat16
    f32 = mybir.dt.float32
    f32r = mybir.dt.float32r
    i16 = mybir.dt.int16

    nc = bacc.Bacc("TRN2", target_bir_lowering=False, debug=False,
                   num_devices=cfg.n_cores, num_swdge_queues=2)

    xTf = nc.dram_tensor("xTf", [D_IN, npad], f32r, kind="ExternalInput")
    AB_w = nc.dram_tensor("AB_w", [D_IN, 320], f32r, kind="ExternalInput")
    eleT = nc.dram_tensor("eleT", [FC_IN, ep], bf, kind="ExternalInput")
    attr_p = nc.dram_tensor("attr_p", [wpc, P, 4 * tt], f32, kind="ExternalInput")
    attr8_p = nc.dram_tensor("attr8_p", [wpc, P, 24 * tt], bf, kind="ExternalInput")
    idx_p = nc.dram_tensor("idx_p", [wpc, 2, P, ni16], i16, kind="ExternalInput")
    ldst_p = nc.dram_tensor("ldst_p", [wpc, P, tt], bf, kind="ExternalInput")
    fcW1 = nc.dram_tensor("fcW1", [FC_IN, FC_H], bf, kind="ExternalInput")
    fcW2 = nc.dram_tensor("fcW2", [FC_H, WN], bf, kind="ExternalInput")
    Wl2_0c = nc.dram_tensor("Wl2_0c", [96, MUL0], bf, kind="ExternalInput")
    Wl2_1c = nc.dram_tensor("Wl2_1c", [96, MUL1], bf, kind="ExternalInput")
    out_d = nc.dram_tensor("out", [npad, D_IN], f32, kind="ExternalOutput")

    y_table = nc.dram_tensor("y_table", [ntab, DE], bf, addr_space="Shared")
    if cfg.use_allgather:
        y_bounce = nc.dram_tensor("y_bounce", [npad, DE], bf)
    else:
        xTbf = nc.dram_tensor("xTbf", [D_IN, ntab], bf, kind="ExternalInput")
        Wl1b = nc.dram_tensor("Wl1b", [D_IN, D_IN], bf, kind="ExternalInput")

    if cfg.null:
        with tile.TileContext(nc) as tc:
            with tc.tile_pool(name="nullp", bufs=1) as npool:
                tnull = npool.tile([P, D_IN], f32)
                nc.gpsimd.memset(tnull[:], 0.0)
                nc.sync.dma_start(out=tnull[:, 0:4], in_=attr_p[0, :, 0:4])
                nc.vector.tensor_scalar(out=tnull[:], in0=tnull[:], scalar1=0.0,
                                        scalar2=None, op0=mybir.AluOpType.mult)
                nc.sync.dma_start(out=out_d[0:P, :], in_=tnull[:])
        nc.compile()
        _PROG_CACHE[cfg] = nc
        return nc

    with tile.TileContext(nc) as tc:
        with (
            tc.tile_pool(name="const", bufs=1) as cpool,
            tc.tile_pool(name="work", bufs=2) as wp,
            tc.tile_pool(name="we", bufs=3) as we,
            tc.tile_pool(name="msgp", bufs=3) as mp,
        ):
            # ---- constants
            iota_i = cpool.tile([P, P], mybir.dt.int32)
            nc.gpsimd.iota(iota_i[:], pattern=[[1, P]], base=0, channel_multiplier=0)
            iota_bf = cpool.tile([P, P], bf)
            nc.vector.tensor_copy(out=iota_bf[:], in_=iota_i[:])
            ident = cpool.tile([P, P], bf)
            make_identity(nc, ident[:])

            fcW1_sb = cpool.tile([FC_IN, FC_H], bf)
            nc.sync.dma_start(out=fcW1_sb[:], in_=fcW1[:, :])
            fcW2_sb = cpool.tile([FC_H, WN], bf)
            nc.sync.dma_start(out=fcW2_sb[:], in_=fcW2[:, :])
            Wl20_sb = cpool.tile([96, MUL0], bf)
            nc.sync.dma_start(out=Wl20_sb[:], in_=Wl2_0c[:, :])
            Wl21_sb = cpool.tile([96, MUL1], bf)
            nc.sync.dma_start(out=Wl21_sb[:], in_=Wl2_1c[:, :])

            AB0 = cpool.tile([P, 320], f32r)
            nc.sync.dma_start(out=AB0[:], in_=AB_w[0:P, :])
            AB1 = cpool.tile([D_IN - P, 320], f32r)
            nc.sync.dma_start(out=AB1[:], in_=AB_w[P:D_IN, :])
            if not cfg.use_allgather:
                Wl1b0 = cpool.tile([P, D_IN], bf)
                nc.sync.dma_start(out=Wl1b0[:], in_=Wl1b[0:P, :])
                Wl1b1 = cpool.tile([D_IN - P, D_IN], bf)
                nc.sync.dma_start(out=Wl1b1[:], in_=Wl1b[P:D_IN, :])

            s_store = cpool.tile([P, wpc * D_IN], f32)

            # ---- phase A: s (self-connection) + local y slice
            psA = ctxA = tc.tile_pool(name="psA", bufs=2, space="PSUM")
            psA = psA.__enter__()
            scols = 320 if cfg.use_allgather else D_IN
            for w in range(wpc):
                if w % 2 == 0:
                    wn = min(2, wpc - w)
                    xa = wp.tile([P, 2 * P], f32r, tag="xa")
                    nc.sync.dma_start(out=xa[:, 0:wn * P],
                                      in_=xTf[0:P, w * P:(w + wn) * P])
                    xb = wp.tile([D_IN - P, 2 * P], f32r, tag="xb")
                    nc.sync.dma_start(out=xb[:, 0:wn * P],
                                      in_=xTf[P:D_IN, w * P:(w + wn) * P])
                o2 = (w % 2) * P
                sy = psA.tile([P, 320], f32, tag="sy")
                nc.tensor.matmul(out=sy[:, 0:scols], lhsT=xa[:, o2:o2 + P],
                                 rhs=AB0[:, 0:scols], start=True, stop=False)
                nc.tensor.matmul(out=sy[:, 0:scols], lhsT=xb[:, o2:o2 + P],
                                 rhs=AB1[:, 0:scols], start=False, stop=True)
                nc.scalar.activation(out=s_store[:, w * D_IN:(w + 1) * D_IN],
                                      in_=sy[:, 0:D_IN],
                                      func=mybir.ActivationFunctionType.Copy)
                if cfg.use_allgather:
                    y_sb = wp.tile([P, D_IN], bf, tag="ysb")
                    nc.scalar.activation(out=y_sb[:], in_=sy[:, D_IN:2 * D_IN],
                                         func=mybir.ActivationFunctionType.Copy)
                    nc.sync.dma_start(out=y_bounce[w * P:(w + 1) * P, 0:D_IN],
                                      in_=y_sb[:])

            if cfg.use_allgather and cfg.n_cores > 1:
                nc.gpsimd.collective_compute(
                    "AllGather",
                    mybir.AluOpType.bypass,
                    replica_groups=[list(range(cfg.n_cores))],
                    ins=[y_bounce[:, :]],
                    outs=[y_table[:, :]],
                )
            elif not cfg.use_allgather:
                gwc = cfg.n_cores * wpc
                GB = 4
                for g0 in range(0, gwc, GB):
                    gb = min(GB, gwc - g0)
                    xab = wp.tile([P, GB * P], bf, tag="xab")
                    nc.sync.dma_start(out=xab[:, 0:gb * P],
                                      in_=xTbf[0:P, g0 * P:(g0 + gb) * P])
                    xbb = wp.tile([D_IN - P, GB * P], bf, tag="xbb")
                    nc.sync.dma_start(out=xbb[:, 0:gb * P],
                                      in_=xTbf[P:D_IN, g0 * P:(g0 + gb) * P])
                    for j in range(gb):
                        yp_full = psA.tile([P, 320], f32, tag="sy")
                        yp = yp_full[:, 0:D_IN]
                        nc.tensor.matmul(out=yp, lhsT=xab[:, j * P:(j + 1) * P],
                                         rhs=Wl1b0[:], start=True, stop=False)
                        nc.tensor.matmul(out=yp, lhsT=xbb[:, j * P:(j + 1) * P],
                                         rhs=Wl1b1[:], start=False, stop=True)
                        yb2 = wp.tile([P, DE], bf, tag="ysb")
                        nc.vector.tensor_copy(out=yb2[:, 0:D_IN], in_=yp)
                        nc.sync.dma_start(
                            out=y_table[(g0 + j) * P:(g0 + j + 1) * P, :],
                            in_=yb2[:])

            ctxA.__exit__(None, None, None)
            psE = ctxE = tc.tile_pool(name="psE", bufs=2, space="PSUM")
            psE = psE.__enter__()
            psZ = ctxZ = tc.tile_pool(name="psZ", bufs=2, space="PSUM")
            psZ = psZ.__enter__()
            psN = ctxN = tc.tile_pool(name="psN", bufs=2, space="PSUM")
            psN = psN.__enter__()
            # ---- edge + node phases
            MU = mybir.AluOpType.mult
            AD = mybir.AluOpType.add
            EQ = mybir.AluOpType.is_equal
            for w in [w_ for _r in range(cfg.repeat) for w_ in range(wpc)]:
                idx_sb = we.tile([P, 2 * ni16], i16, tag="idx")
                nc.sync.dma_start(out=idx_sb[:, 0:ni16], in_=idx_p[w, 0, :, :])
                nc.sync.dma_start(out=idx_sb[:, ni16:2 * ni16], in_=idx_p[w, 1, :, :])
                ldstf_sb = we.tile([P, tt], bf, tag="ldst")
                nc.sync.dma_start(out=ldstf_sb[:], in_=ldst_p[w, :, :])
                at_sb = we.tile([P, 4 * tt], f32, tag="attr")
                nc.sync.dma_start(out=at_sb[:], in_=attr_p[w, :, :])
                a8_sb = we.tile([P, 24 * tt], bf, tag="attr8")
                nc.sync.dma_start(out=a8_sb[:], in_=attr8_p[w, :, :])
                el_sb = we.tile([FC_IN, tt * P], bf, tag="ele")
                nc.sync.dma_start(out=el_sb[:],
                                  in_=eleT[:, w * tt * P:(w + 1) * tt * P])
                ys_all = we.tile([P, tt * DE], bf, tag="ys")
                if not cfg.ab_no_gather:
                    nc.gpsimd.dma_gather(
                        out_ap=ys_all[:, 0:th * DE].rearrange("p (t f) -> p t f", f=DE),
                        in_ap=y_table[0:half, :],
                        idxs_ap=idx_sb[:, 0:ni16],
                        num_idxs=ni, num_idxs_reg=ni, elem_size=DE,
                        single_packet=False)
                    nc.gpsimd.dma_gather(
                        out_ap=ys_all[:, th * DE:tt * DE].rearrange("p (t f) -> p t f", f=DE),
                        in_ap=y_table[half:ntab, :],
                        idxs_ap=idx_sb[:, ni16:2 * ni16],
                        num_idxs=ni, num_idxs_reg=ni, elem_size=DE,
                        single_packet=False, queue_num=1)

                z_ps = psZ.tile([P, D_MID], f32, tag="z")
                ysv = ys_all[:].rearrange("p (t f) -> p t f", f=DE)
                for g0 in range(0, tt, 8):
                    gs = min(8, tt - g0)
                    # FC in 4-tile chunks; per-edge weights into w4_sb (8 tiles)
                    w4_sb = mp.tile([P, gs * WN], bf, tag="w4")
                    for f0 in ([] if cfg.ab_no_fc else range(0, gs, 4)):
                        fs = min(4, gs - f0)
                        hT_ps = psE.tile([FC_H, fs * P], f32, tag="ht")
                        nc.tensor.matmul(out=hT_ps[:], lhsT=fcW1_sb[:],
                                         rhs=el_sb[:, (g0 + f0) * P:(g0 + f0 + fs) * P],
                                         start=True, stop=True)
                        hT_sb = mp.tile([FC_H, fs * P], bf, tag="hts")
                        nc.scalar.activation(out=hT_sb[:], in_=hT_ps[:],
                                             func=mybir.ActivationFunctionType.Silu)
                        for p0 in range(0, fs, 2):
                            w2_ps = psE.tile([P, 2 * WN], f32, tag="w")
                            for j in range(2):
                                nc.tensor.matmul(
                                    out=w2_ps[:, j * WN:(j + 1) * WN],
                                    lhsT=hT_sb[:, (p0 + j) * P:(p0 + j + 1) * P],
                                    rhs=fcW2_sb[:], start=True, stop=True)
                            nc.scalar.activation(
                                out=w4_sb[:, (f0 + p0) * WN:(f0 + p0 + 2) * WN],
                                in_=w2_ps[:],
                                func=mybir.ActivationFunctionType.Copy)

                    # selection matrices (per tile, 4x tensor_scalar)
                    S4 = mp.tile([P, gs * P], bf, tag="S")
                    if not cfg.ab_no_s4:
                        nc.vector.tensor_tensor(
                            out=S4[:].rearrange("p (t q) -> p t q", q=P),
                            in0=iota_bf[:].unsqueeze(1).broadcast_to([P, gs, P]),
                            in1=ldstf_sb[:, g0:g0 + gs].unsqueeze(2)
                                .broadcast_to([P, gs, P]),
                            op=EQ)

                    # message build, batched across gs tiles
                    at4 = at_sb[:, 4 * g0:4 * (g0 + gs)].rearrange(
                        "p (t c) -> p t c", c=4)
                    a84 = a8_sb[:, 24 * g0:24 * (g0 + gs)].rearrange(
                        "p (t c r) -> p t c r", c=3, r=8)
                    ysg = ysv[:, g0:g0 + gs, :]
                    ys0v = ysg[:, :, 0:MUL0]
                    ys1v = ysg[:, :, MUL0:D_IN].rearrange(
                        "p t (c u) -> p t c u", u=MUL1)
                    w4v = w4_sb[:].rearrange("p (t k) -> p t k", k=WN)
                    msg4 = mp.tile([P, gs * D_MID], bf, tag="msg")
                    msgv = msg4[:].rearrange("p (t k) -> p t k", k=D_MID)
                    msg1 = msgv[:, :, 96:D_MID].rearrange(
                        "p t (c x) -> p t c x", x=96)

                    vtt = ((lambda **kw: None) if cfg.ab_no_msg
                           else nc.vector.tensor_tensor)
                    tw = mp.tile([P, gs * 2 * MUL0], bf, tag="tw")
                    twv = tw[:].rearrange("p (t r u) -> p t r u", r=2, u=MUL0)
                    vtt(
                        out=twv,
                        in0=w4v[:, :, 0:2 * MUL0].rearrange(
                            "p t (r u) -> p t r u", u=MUL0),
                        in1=ys0v.unsqueeze(2).broadcast_to([P, gs, 2, MUL0]),
                        op=MU)
                    vtt(
                        out=msgv[:, :, 0:MUL0],
                        in0=twv[:, :, 0, :],
                        in1=at4[:, :, 0:1].broadcast_to([P, gs, MUL0]),
                        op=MU)
                    dm = mp.tile([P, gs * 96], bf, tag="dm")
                    dmv = dm[:].rearrange("p (t c u) -> p t c u", c=3, u=MUL1)
                    vtt(
                        out=dmv.rearrange("p t c (v r) -> p t c v r", r=8),
                        in0=ys1v.rearrange("p t c (v r) -> p t c v r", r=8),
                        in1=a84.unsqueeze(3).broadcast_to([P, gs, 3, MUL1 // 8, 8]),
                        op=MU)
                    ds = mp.tile([P, gs * MUL1], bf, tag="ds")
                    dsv = ds[:].rearrange("p (t u) -> p t u", u=MUL1)
                    vtt(out=dsv, in0=dmv[:, :, 0, :],
                                            in1=dmv[:, :, 1, :], op=AD)
                    vtt(out=dsv, in0=dsv, in1=dmv[:, :, 2, :],
                                            op=AD)
                    vtt(out=msgv[:, :, MUL0:96], in0=dsv,
                                            in1=w4v[:, :, 160:WN], op=MU)
                    t3t = mp.tile([P, gs * MUL1], bf, tag="t3")
                    t3v = t3t[:].rearrange("p (t u) -> p t u", u=MUL1)
                    vtt(
                        out=t3v, in0=w4v[:, :, 128:160],
                        in1=at4[:, :, 0:1].broadcast_to([P, gs, MUL1]), op=MU)
                    vtt(
                        out=msg1[:, :, :, 0:MUL0].rearrange(
                            "p t c (v r) -> p t c v r", r=8),
                        in0=twv[:, :, 1:2, :].broadcast_to(
                            [P, gs, 3, MUL0]).rearrange(
                            "p t c (v r) -> p t c v r", r=8),
                        in1=a84.unsqueeze(3).broadcast_to([P, gs, 3, MUL0 // 8, 8]),
                        op=MU)
                    vtt(
                        out=msg1[:, :, :, MUL0:96],
                        in0=t3v.unsqueeze(2).broadcast_to([P, gs, 3, MUL1]),
                        in1=ys1v, op=MU)

                    for j in range(gs):
                        t = g0 + j
                        if cfg.ab_no_sel and t > 0:
                            continue
                        nc.tensor.matmul(
                            out=z_ps[:], lhsT=S4[:, j * P:(j + 1) * P],
                            rhs=msg4[:, j * D_MID:(j + 1) * D_MID],
                            start=(t == 0),
                            stop=(t == tt - 1) or cfg.ab_no_sel)

                # ---- node phase for this window
                if cfg.ab_no_node:
                    out_sb = mp.tile([P, D_IN], f32, tag="outsb")
                    nc.vector.tensor_copy(
                        out=out_sb[:], in_=s_store[:, w * D_IN:(w + 1) * D_IN])
                    nc.sync.dma_start(out=out_d[w * P:(w + 1) * P, :],
                                      in_=out_sb[:])
                    continue
                z_sb = mp.tile([P, D_MID], bf, tag="zsb")
                nc.scalar.activation(out=z_sb[:], in_=z_ps[:],
                                     func=mybir.ActivationFunctionType.Copy)
                o_ps = psN.tile([P, D_IN], f32, tag="o")
                for b in range(4):
                    zT_ps = psN.tile([96, P], bf, tag="o")
                    nc.tensor.transpose(out=zT_ps[:], in_=z_sb[:, b * 96:(b + 1) * 96],
                                        identity=ident[:])
                    zT_sb = mp.tile([96, P], bf, tag="zts")
                    nc.scalar.activation(out=zT_sb[:], in_=zT_ps[:],
                                         func=mybir.ActivationFunctionType.Copy)
                    if b == 0:
                        nc.tensor.matmul(out=o_ps[:, 0:MUL0], lhsT=zT_sb[:],
                                         rhs=Wl20_sb[:], start=True, stop=True)
                    else:
                        c = b - 1
                        nc.tensor.matmul(
                            out=o_ps[:, MUL0 + c * MUL1:MUL0 + (c + 1) * MUL1],
                            lhsT=zT_sb[:], rhs=Wl21_sb[:], start=True, stop=True)
                out_sb = mp.tile([P, D_IN], f32, tag="outsb")
                nc.vector.tensor_tensor(out=out_sb[:], in0=o_ps[:],
                                        in1=s_store[:, w * D_IN:(w + 1) * D_IN],
                                        op=AD)
                nc.sync.dma_start(out=out_d[w * P:(w + 1) * P, :], in_=out_sb[:])
            ctxN.__exit__(None, None, None)
            ctxZ.__exit__(None, None, None)
            ctxE.__exit__(None, None, None)

    nc.compile()
    _PROG_CACHE[cfg] = nc
    return nc


# ---------------------------------------------------------------- entry point

def _assemble(cfg: Cfg, results, node_attr):
    outs = [results[k]["out"][:cfg.npc] for k in range(cfg.n_cores)]
    o = np.concatenate(outs, axis=0).astype(F32)
    o = np.concatenate([o[:, :MUL0], _from_cmajor(o[:, MUL0:])], axis=1)
    return o * node_attr


def kernel(**inputs):
    cfg, in_maps, node_attr = _prep(inputs, n_cores=8)
    nc = _build(cfg)
    res = run_bass_kernel_spmd(nc, in_maps, core_ids=list(range(cfg.n_cores)))
    return _assemble(cfg, res.results, node_attr)



# revision 7
# speedup vs baseline: 84.8369x; 84.8369x over previous
"""Trainium2 Bass kernel for the e3nn-style GNN convolution layer.

kernel(**inputs) takes FULL (unsharded) numpy inputs and returns the FULL
[N, 160] float32 output.  Internally shards edges across 8 NeuronCores by
destination-node range, runs one SPMD Bass program, and reassembles on host.

Design:
  host prep   - fold all scalar normalizations into weights; x~ = node_input *
                node_attr; transpose node features (1o block c-major); sort
                edges by (dst-window, src-table-half), pad each half-group to
                a fixed number of 128-edge tiles (padding has edge_attr = 0 so
                its messages vanish).
  phase A     - per 128-node window: one fused [s | y] float32r matmul
                (self-connection s kept in SBUF, lin1 output y written bf16);
                AllGather replicates the y table across the 8 cores.
  edge phase  - per window: 2 dma_gather calls fetch y[src] rows (512B each)
                from the two table halves; per 128-edge tile: FC net (2 bf16
                matmuls + Silu), message build on DVE/ACT, and a one-hot
                selection matmul accumulating z in PSUM.
  node phase  - per window: transpose z (PE), lin2 (4 bf16 matmuls), add s,
                write the output slice.
"""

import math
from dataclasses import dataclass

import numpy as np
import ml_dtypes

import concourse.bacc as bacc
import concourse.bass as bass
import concourse.mybir as mybir
import concourse.tile as tile
from concourse.bass_utils import run_bass_kernel_spmd
from concourse.masks import make_identity

USE_ALLGATHER = False
BF16 = ml_dtypes.bfloat16
F32 = np.float32

MUL0 = 64
MUL1 = 32
FC_IN = 16
FC_H = 64
WN = 2 * MUL0 + 2 * MUL1  # 192 per-edge tp weights
D_IN = MUL0 + 3 * MUL1    # 160
DE = 256                  # padded y-table row elems (512 B in bf16)
D_MID = 4 * (MUL0 + MUL1) # 384 = [z0 (96) | z1_c0 | z1_c1 | z1_c2]
NUM_NEIGH = 10.0
C_S = math.sin(math.pi / 8.0)
C_X = math.cos(math.pi / 8.0)
P = 128


@dataclass(frozen=True)
class Cfg:
    n: int
    n_cores: int
    npc: int          # nodes per core
    wpc: int          # 128-node windows per core
    npad: int         # wpc * 128
    th: int           # tiles per (window, table-half)
    use_allgather: bool = True
    repeat: int = 1
    null: bool = False
    # ablation flags (timing experiments only; break correctness)
    ab_no_gather: bool = False
    ab_no_fc: bool = False
    ab_no_msg: bool = False
    ab_no_sel: bool = False
    ab_no_s4: bool = False
    ab_no_node: bool = False
    sim_1q: bool = False  # force both gathers onto swdge queue 0 (sim only)


def _to_cmajor(x_uc):
    s = x_uc.shape[:-1]
    return x_uc.reshape(*s, MUL1, 3).swapaxes(-1, -2).reshape(*s, 96)


def _from_cmajor(x_cu):
    s = x_cu.shape[:-1]
    return x_cu.reshape(*s, 3, MUL1).swapaxes(-1, -2).reshape(*s, 96)


# ---------------------------------------------------------------- host prep

def _prep(inputs, n_cores=8):
    node_input = np.asarray(inputs["node_input"], F32)
    node_attr = np.asarray(inputs["node_attr"], F32)
    edge_src = np.asarray(inputs["edge_src"]).astype(np.int64)
    edge_dst = np.asarray(inputs["edge_dst"]).astype(np.int64)
    edge_attr = np.asarray(inputs["edge_attr"], F32)
    ele = np.asarray(inputs["edge_length_embedded"], F32)

    n = node_input.shape[0]
    e = edge_src.shape[0]
    assert n % n_cores == 0
    npc = n // n_cores
    wpc = (npc + P - 1) // P
    npad = wpc * P
    ntab = n_cores * npad
    half = ntab // 2
    assert half <= 32767 and half % npad == 0

    inv0 = 1.0 / math.sqrt(MUL0)
    inv1 = 1.0 / math.sqrt(MUL1)
    invm = 1.0 / math.sqrt(MUL0 + MUL1)
    invnb = 1.0 / math.sqrt(NUM_NEIGH)

    x = node_input * node_attr
    xT = np.concatenate([x[:, :MUL0], _to_cmajor(x[:, MUL0:])], axis=1).T  # [160,n]
    xT = np.ascontiguousarray(xT, F32)

    W_sc0 = np.asarray(inputs["W_sc0"], F32) * (inv0 * C_S)
    W_sc1 = np.asarray(inputs["W_sc1"], F32) * (inv1 * C_S)
    W_l1_0 = np.asarray(inputs["W_l1_0"], F32) * inv0
    W_l1_1 = np.asarray(inputs["W_l1_1"], F32) * inv1
    fc_W1 = np.asarray(inputs["fc_W1"], F32) * (1.0 / math.sqrt(FC_IN))
    fc_W2 = np.asarray(inputs["fc_W2"], F32) * (1.0 / math.sqrt(FC_H))
    obase = invm * C_X * invnb
    W_l2_0 = np.asarray(inputs["W_l2_0"], F32) * obase
    W_l2_0 = W_l2_0.copy()
    W_l2_0[MUL0:, :] *= 1.0 / math.sqrt(3.0)
    W_l2_1 = np.asarray(inputs["W_l2_1"], F32) * obase

    def blockdiag(*ms):
        rows = sum(m.shape[0] for m in ms)
        cols = sum(m.shape[1] for m in ms)
        out = np.zeros((rows, cols), F32)
        r = c = 0
        for m in ms:
            out[r:r + m.shape[0], c:c + m.shape[1]] = m
            r += m.shape[0]
            c += m.shape[1]
        return out

    Wsc_big = blockdiag(W_sc0, W_sc1, W_sc1, W_sc1)
    Wl1_big = blockdiag(W_l1_0, W_l1_1, W_l1_1, W_l1_1)
    AB = np.ascontiguousarray(np.concatenate([Wsc_big, Wl1_big], axis=1), F32)

    # ---- edge sharding: (dst-window, src-half) groups
    core = edge_dst // npc
    local = edge_dst - core * npc
    win = local // P
    ldst = (local - win * P).astype(F32)
    src_remap = (edge_src // npc) * npad + (edge_src % npc)
    hbit = (src_remap >= half).astype(np.int64)
    g2 = (core * wpc + win) * 2 + hbit              # (window, half) group id
    order = np.argsort(g2, kind="stable")
    cnt2 = np.bincount(g2, minlength=n_cores * wpc * 2)
    th = max(1, int((cnt2.max() + P - 1) // P))     # tiles per half-group
    ni = th * P                                     # slots per half-group
    epw = 2 * ni                                    # edge slots per window
    tt = 2 * th                                     # tiles per window

    starts = np.zeros(n_cores * wpc * 2, np.int64)
    starts[1:] = np.cumsum(cnt2)[:-1]
    j_within = np.arange(e) - starts[g2[order]]
    dest = g2[order] * ni + j_within                # flat padded slot

    flat = n_cores * wpc * epw
    A_ = np.zeros((flat, 4), F32)
    A_[dest] = edge_attr[order]
    IDX = np.zeros(flat, np.int16)
    IDX[dest] = (src_remap[order] - hbit[order] * half).astype(np.int16)
    L_ = np.zeros(flat, F32)
    L_[dest] = ldst[order]
    E_ = np.zeros((flat, FC_IN), F32)
    E_[dest] = ele[order]

    E1_ = A_[:, 1:4]                                  # [flat, 3] (pre-transpose)
    A_ = A_.reshape(n_cores, wpc, tt, P, 4).transpose(0, 1, 3, 2, 4)
    attr_p = np.ascontiguousarray(A_.reshape(n_cores, wpc, P, tt * 4), F32)
    E8 = np.repeat(E1_[:, :, None], 8, axis=2)        # [flat, 3, 8]
    E8 = E8.reshape(n_cores, wpc, tt, P, 24).transpose(0, 1, 3, 2, 4)
    attr8_p = np.ascontiguousarray(E8.reshape(n_cores, wpc, P, tt * 24), BF16)
    ldst_p = np.ascontiguousarray(
        L_.reshape(n_cores, wpc, tt, P).transpose(0, 1, 3, 2), BF16)
    eleT_p = np.ascontiguousarray(
        E_.reshape(n_cores, wpc * tt * P, FC_IN).transpose(0, 2, 1), BF16)
    # idx wrapped for dma_gather: j -> (j%16, j//16), replicated over 8 groups
    ni16 = ni // 16
    IW = IDX.reshape(n_cores, wpc, 2, ni16, 16).swapaxes(3, 4)  # [c,w,h,16,ni16]
    idx_p = np.ascontiguousarray(
        np.broadcast_to(IW[:, :, :, None, :, :],
                        (n_cores, wpc, 2, 8, 16, ni16))
        .reshape(n_cores, wpc, 2, P, ni16))

    xT_pad = np.zeros((n_cores, D_IN, npad), F32)
    for k in range(n_cores):
        xT_pad[k, :, :npc] = xT[:, k * npc:(k + 1) * npc]
    xTbf_full = np.ascontiguousarray(
        xT_pad.transpose(1, 0, 2).reshape(D_IN, ntab), BF16)

    cfg = Cfg(n=n, n_cores=n_cores, npc=npc, wpc=wpc, npad=npad, th=th,
              use_allgather=USE_ALLGATHER)

    in_maps = []
    for k in range(n_cores):
        m = {
            "xTf": np.ascontiguousarray(xT_pad[k]),
            "AB_w": AB,
            "eleT": eleT_p[k],
            "attr_p": attr_p[k],
            "attr8_p": attr8_p[k],
            "idx_p": idx_p[k],
            "ldst_p": ldst_p[k],
            "fcW1": np.ascontiguousarray(fc_W1, BF16),
            "fcW2": np.ascontiguousarray(fc_W2, BF16),
            "Wl2_0c": np.ascontiguousarray(W_l2_0, BF16),
            "Wl2_1c": np.ascontiguousarray(W_l2_1, BF16),
        }
        if not cfg.use_allgather:
            m["xTbf"] = xTbf_full
            m["Wl1b"] = np.ascontiguousarray(Wl1_big, BF16)
        in_maps.append(m)
    return cfg, in_maps, node_attr


# ---------------------------------------------------------------- device program

_PROG_CACHE = {}


def _build(cfg: Cfg):
    if cfg in _PROG_CACHE:
        return _PROG_CACHE[cfg]

    th, wpc, npad = cfg.th, cfg.wpc, cfg.npad
    tt = 2 * th
    ni = th * P
    ni16 = ni // 16
    ep = wpc * tt * P
    ntab = cfg.n_cores * npad
    half = ntab // 2
    bf = mybir.dt.bfloat16
    f32 = mybir.dt.float32
    f32r = mybir.dt.float32r
    i16 = mybir.dt.int16

    nc = bacc.Bacc("TRN2", target_bir_lowering=False, debug=False,
                   num_devices=cfg.n_cores, num_swdge_queues=2)

    xTf = nc.dram_tensor("xTf", [D_IN, npad], f32r, kind="ExternalInput")
    AB_w = nc.dram_tensor("AB_w", [D_IN, 320], f32r, kind="ExternalInput")
    eleT = nc.dram_tensor("eleT", [FC_IN, ep], bf, kind="ExternalInput")
    attr_p = nc.dram_tensor("attr_p", [wpc, P, 4 * tt], f32, kind="ExternalInput")
    attr8_p = nc.dram_tensor("attr8_p", [wpc, P, 24 * tt], bf, kind="ExternalInput")
    idx_p = nc.dram_tensor("idx_p", [wpc, 2, P, ni16], i16, kind="ExternalInput")
    ldst_p = nc.dram_tensor("ldst_p", [wpc, P, tt], bf, kind="ExternalInput")
    fcW1 = nc.dram_tensor("fcW1", [FC_IN, FC_H], bf, kind="ExternalInput")
    fcW2 = nc.dram_tensor("fcW2", [FC_H, WN], bf, kind="ExternalInput")
    Wl2_0c = nc.dram_tensor("Wl2_0c", [96, MUL0], bf, kind="ExternalInput")
    Wl2_1c = nc.dram_tensor("Wl2_1c", [96, MUL1], bf, kind="ExternalInput")
    out_d = nc.dram_tensor("out", [npad, D_IN], f32, kind="ExternalOutput")

    y_table = nc.dram_tensor("y_table", [ntab, DE], bf, addr_space="Shared")
    if cfg.use_allgather:
        y_bounce = nc.dram_tensor("y_bounce", [npad, DE], bf)
    else:
        xTbf = nc.dram_tensor("xTbf", [D_IN, ntab], bf, kind="ExternalInput")
        Wl1b = nc.dram_tensor("Wl1b", [D_IN, D_IN], bf, kind="ExternalInput")

    if cfg.null:
        with tile.TileContext(nc) as tc:
            with tc.tile_pool(name="nullp", bufs=1) as npool:
                tnull = npool.tile([P, D_IN], f32)
                nc.gpsimd.memset(tnull[:], 0.0)
                nc.sync.dma_start(out=tnull[:, 0:4], in_=attr_p[0, :, 0:4])
                nc.vector.tensor_scalar(out=tnull[:], in0=tnull[:], scalar1=0.0,
                                        scalar2=None, op0=mybir.AluOpType.mult)
                nc.sync.dma_start(out=out_d[0:P, :], in_=tnull[:])
        nc.compile()
        _PROG_CACHE[cfg] = nc
        return nc

    with tile.TileContext(nc) as tc:
        with (
            tc.tile_pool(name="const", bufs=1) as cpool,
            tc.tile_pool(name="work", bufs=2) as wp,
            tc.tile_pool(name="we", bufs=3) as we,
            tc.tile_pool(name="msgp", bufs=3) as mp,
        ):
            # ---- constants
            iota_i = cpool.tile([P, P], mybir.dt.int32)
            nc.gpsimd.iota(iota_i[:], pattern=[[1, P]], base=0, channel_multiplier=0)
            iota_bf = cpool.tile([P, P], bf)
            nc.vector.tensor_copy(out=iota_bf[:], in_=iota_i[:])
            ident = cpool.tile([P, P], bf)
            make_identity(nc, ident[:])

            fcW1_sb = cpool.tile([FC_IN, FC_H], bf)
            nc.sync.dma_start(out=fcW1_sb[:], in_=fcW1[:, :])
            fcW2_sb = cpool.tile([FC_H, WN], bf)
            nc.sync.dma_start(out=fcW2_sb[:], in_=fcW2[:, :])
            Wl20_sb = cpool.tile([96, MUL0], bf)
            nc.sync.dma_start(out=Wl20_sb[:], in_=Wl2_0c[:, :])
            Wl21_sb = cpool.tile([96, MUL1], bf)
            nc.sync.dma_start(out=Wl21_sb[:], in_=Wl2_1c[:, :])

            AB0 = cpool.tile([P, 320], f32r)
            nc.sync.dma_start(out=AB0[:], in_=AB_w[0:P, :])
            AB1 = cpool.tile([D_IN - P, 320], f32r)
            nc.sync.dma_start(out=AB1[:], in_=AB_w[P:D_IN, :])
            if not cfg.use_allgather:
                Wl1b0 = cpool.tile([P, D_IN], bf)
                nc.sync.dma_start(out=Wl1b0[:], in_=Wl1b[0:P, :])
                Wl1b1 = cpool.tile([D_IN - P, D_IN], bf)
                nc.sync.dma_start(out=Wl1b1[:], in_=Wl1b[P:D_IN, :])

            s_store = cpool.tile([P, wpc * D_IN], f32)

            # ---- phase A: s (self-connection) + local y slice
            psA = ctxA = tc.tile_pool(name="psA", bufs=2, space="PSUM")
            psA = psA.__enter__()
            scols = 320 if cfg.use_allgather else D_IN
            for w in range(wpc):
                if w % 2 == 0:
                    wn = min(2, wpc - w)
                    xa = wp.tile([P, 2 * P], f32r, tag="xa")
                    nc.sync.dma_start(out=xa[:, 0:wn * P],
                                      in_=xTf[0:P, w * P:(w + wn) * P])
                    xb = wp.tile([D_IN - P, 2 * P], f32r, tag="xb")
                    nc.sync.dma_start(out=xb[:, 0:wn * P],
                                      in_=xTf[P:D_IN, w * P:(w + wn) * P])
                o2 = (w % 2) * P
                sy = psA.tile([P, 320], f32, tag="sy")
                nc.tensor.matmul(out=sy[:, 0:scols], lhsT=xa[:, o2:o2 + P],
                                 rhs=AB0[:, 0:scols], start=True, stop=False)
                nc.tensor.matmul(out=sy[:, 0:scols], lhsT=xb[:, o2:o2 + P],
                                 rhs=AB1[:, 0:scols], start=False, stop=True)
                nc.scalar.activation(out=s_store[:, w * D_IN:(w + 1) * D_IN],
                                      in_=sy[:, 0:D_IN],
                                      func=mybir.ActivationFunctionType.Copy)
                if cfg.use_allgather:
                    y_sb = wp.tile([P, D_IN], bf, tag="ysb")
                    nc.scalar.activation(out=y_sb[:], in_=sy[:, D_IN:2 * D_IN],
                                         func=mybir.ActivationFunctionType.Copy)
                    nc.sync.dma_start(out=y_bounce[w * P:(w + 1) * P, 0:D_IN],
                                      in_=y_sb[:])

            if cfg.use_allgather and cfg.n_cores > 1:
                nc.gpsimd.collective_compute(
                    "AllGather",
                    mybir.AluOpType.bypass,
                    replica_groups=[list(range(cfg.n_cores))],
                    ins=[y_bounce[:, :]],
                    outs=[y_table[:, :]],
                )
            elif not cfg.use_allgather:
                gwc = cfg.n_cores * wpc
                GB = 4
                for g0 in range(0, gwc, GB):
                    gb = min(GB, gwc - g0)
                    xab = wp.tile([P, GB * P], bf, tag="xab")
                    nc.sync.dma_start(out=xab[:, 0:gb * P],
                                      in_=xTbf[0:P, g0 * P:(g0 + gb) * P])
                    xbb = wp.tile([D_IN - P, GB * P], bf, tag="xbb")
                    nc.sync.dma_start(out=xbb[:, 0:gb * P],
                                      in_=xTbf[P:D_IN, g0 * P:(g0 + gb) * P])
                    for j in range(gb):
                        yp_full = psA.tile([P, 320], f32, tag="sy")
                        yp = yp_full[:, 0:D_IN]
                        nc.tensor.matmul(out=yp, lhsT=xab[:, j * P:(j + 1) * P],
                                         rhs=Wl1b0[:], start=True, stop=False)
                        nc.tensor.matmul(out=yp, lhsT=xbb[:, j * P:(j + 1) * P],
                                         rhs=Wl1b1[:], start=False, stop=True)
                        yb2 = wp.tile([P, DE], bf, tag="ysb")
                        nc.vector.tensor_copy(out=yb2[:, 0:D_IN], in_=yp)
                        nc.sync.dma_start(
                            out=y_table[(g0 + j) * P:(g0 + j + 1) * P, :],
                            in_=yb2[:])

            ctxA.__exit__(None, None, None)
            psE = ctxE = tc.tile_pool(name="psE", bufs=2, space="PSUM")
            psE = psE.__enter__()
            psZ = ctxZ = tc.tile_pool(name="psZ", bufs=2, space="PSUM")
            psZ = psZ.__enter__()
            psN = ctxN = tc.tile_pool(name="psN", bufs=2, space="PSUM")
            psN = psN.__enter__()
            # ---- edge + node phases
            MU = mybir.AluOpType.mult
            AD = mybir.AluOpType.add
            EQ = mybir.AluOpType.is_equal
            for w in [w_ for _r in range(cfg.repeat) for w_ in range(wpc)]:
                idx_sb = we.tile([P, 2 * ni16], i16, tag="idx")
                nc.sync.dma_start(out=idx_sb[:, 0:ni16], in_=idx_p[w, 0, :, :])
                nc.sync.dma_start(out=idx_sb[:, ni16:2 * ni16], in_=idx_p[w, 1, :, :])
                ldstf_sb = we.tile([P, tt], bf, tag="ldst")
                nc.sync.dma_start(out=ldstf_sb[:], in_=ldst_p[w, :, :])
                at_sb = we.tile([P, 4 * tt], f32, tag="attr")
                nc.sync.dma_start(out=at_sb[:], in_=attr_p[w, :, :])
                a8_sb = we.tile([P, 24 * tt], bf, tag="attr8")
                nc.sync.dma_start(out=a8_sb[:], in_=attr8_p[w, :, :])
                el_sb = we.tile([FC_IN, tt * P], bf, tag="ele")
                nc.sync.dma_start(out=el_sb[:],
                                  in_=eleT[:, w * tt * P:(w + 1) * tt * P])
                ys_all = we.tile([P, tt * DE], bf, tag="ys")
                if cfg.ab_no_gather:
                    nc.gpsimd.memset(ys_all[:, 0:16], 0.0)
                if not cfg.ab_no_gather:
                    nc.gpsimd.dma_gather(
                        out_ap=ys_all[:, 0:th * DE].rearrange("p (t f) -> p t f", f=DE),
                        in_ap=y_table[0:half, :],
                        idxs_ap=idx_sb[:, 0:ni16],
                        num_idxs=ni, num_idxs_reg=ni, elem_size=DE,
                        single_packet=False)
                    nc.gpsimd.dma_gather(
                        out_ap=ys_all[:, th * DE:tt * DE].rearrange("p (t f) -> p t f", f=DE),
                        in_ap=y_table[half:ntab, :],
                        idxs_ap=idx_sb[:, ni16:2 * ni16],
                        num_idxs=ni, num_idxs_reg=ni, elem_size=DE,
                        single_packet=False,
                        queue_num=0 if cfg.sim_1q else 1)

                z_ps = psZ.tile([P, D_MID], f32, tag="z")
                ysv = ys_all[:].rearrange("p (t f) -> p t f", f=DE)
                for g0 in range(0, tt, 8):
                    gs = min(8, tt - g0)
                    # FC in 4-tile chunks; per-edge weights into w4_sb (8 tiles)
                    w4_sb = mp.tile([P, gs * WN], bf, tag="w4")
                    if cfg.ab_no_fc:
                        nc.gpsimd.memset(w4_sb[:, 0:16], 0.0)
                    for f0 in ([] if cfg.ab_no_fc else range(0, gs, 4)):
                        fs = min(4, gs - f0)
                        hT_ps = psE.tile([FC_H, fs * P], f32, tag="ht")
                        nc.tensor.matmul(out=hT_ps[:], lhsT=fcW1_sb[:],
                                         rhs=el_sb[:, (g0 + f0) * P:(g0 + f0 + fs) * P],
                                         start=True, stop=True)
                        hT_sb = mp.tile([FC_H, fs * P], bf, tag="hts")
                        nc.scalar.activation(out=hT_sb[:], in_=hT_ps[:],
                                             func=mybir.ActivationFunctionType.Silu)
                        for p0 in range(0, fs, 2):
                            w2_ps = psE.tile([P, 2 * WN], f32, tag="w")
                            for j in range(2):
                                nc.tensor.matmul(
                                    out=w2_ps[:, j * WN:(j + 1) * WN],
                                    lhsT=hT_sb[:, (p0 + j) * P:(p0 + j + 1) * P],
                                    rhs=fcW2_sb[:], start=True, stop=True)
                            nc.scalar.activation(
                                out=w4_sb[:, (f0 + p0) * WN:(f0 + p0 + 2) * WN],
                                in_=w2_ps[:],
                                func=mybir.ActivationFunctionType.Copy)

                    # selection matrices (per tile, 4x tensor_scalar)
                    S4 = mp.tile([P, gs * P], bf, tag="S")
                    if cfg.ab_no_s4:
                        nc.gpsimd.memset(S4[:, 0:16], 0.0)
                    if not cfg.ab_no_s4:
                        nc.vector.tensor_tensor(
                            out=S4[:].rearrange("p (t q) -> p t q", q=P),
                            in0=iota_bf[:].unsqueeze(1).broadcast_to([P, gs, P]),
                            in1=ldstf_sb[:, g0:g0 + gs].unsqueeze(2)
                                .broadcast_to([P, gs, P]),
                            op=EQ)

                    # message build, batched across gs tiles
                    at4 = at_sb[:, 4 * g0:4 * (g0 + gs)].rearrange(
                        "p (t c) -> p t c", c=4)
                    a84 = a8_sb[:, 24 * g0:24 * (g0 + gs)].rearrange(
                        "p (t c r) -> p t c r", c=3, r=8)
                    ysg = ysv[:, g0:g0 + gs, :]
                    ys0v = ysg[:, :, 0:MUL0]
                    ys1v = ysg[:, :, MUL0:D_IN].rearrange(
                        "p t (c u) -> p t c u", u=MUL1)
                    w4v = w4_sb[:].rearrange("p (t k) -> p t k", k=WN)
                    msg4 = mp.tile([P, gs * D_MID], bf, tag="msg")
                    if cfg.ab_no_msg:
                        nc.gpsimd.memset(msg4[:, 0:16], 0.0)
                    msgv = msg4[:].rearrange("p (t k) -> p t k", k=D_MID)
                    msg1 = msgv[:, :, 96:D_MID].rearrange(
                        "p t (c x) -> p t c x", x=96)

                    vtt = ((lambda **kw: None) if cfg.ab_no_msg
                           else nc.vector.tensor_tensor)
                    tw = mp.tile([P, gs * 2 * MUL0], bf, tag="tw")
                    twv = tw[:].rearrange("p (t r u) -> p t r u", r=2, u=MUL0)
                    vtt(
                        out=twv,
                        in0=w4v[:, :, 0:2 * MUL0].rearrange(
                            "p t (r u) -> p t r u", u=MUL0),
                        in1=ys0v.unsqueeze(2).broadcast_to([P, gs, 2, MUL0]),
                        op=MU)
                    vtt(
                        out=msgv[:, :, 0:MUL0],
                        in0=twv[:, :, 0, :],
                        in1=at4[:, :, 0:1].broadcast_to([P, gs, MUL0]),
                        op=MU)
                    dm = mp.tile([P, gs * 96], bf, tag="dm")
                    dmv = dm[:].rearrange("p (t c u) -> p t c u", c=3, u=MUL1)
                    vtt(
                        out=dmv.rearrange("p t c (v r) -> p t c v r", r=8),
                        in0=ys1v.rearrange("p t c (v r) -> p t c v r", r=8),
                        in1=a84.unsqueeze(3).broadcast_to([P, gs, 3, MUL1 // 8, 8]),
                        op=MU)
                    ds = mp.tile([P, gs * MUL1], bf, tag="ds")
                    dsv = ds[:].rearrange("p (t u) -> p t u", u=MUL1)
                    vtt(out=dsv, in0=dmv[:, :, 0, :],
                                            in1=dmv[:, :, 1, :], op=AD)
                    vtt(out=dsv, in0=dsv, in1=dmv[:, :, 2, :],
                                            op=AD)
                    vtt(out=msgv[:, :, MUL0:96], in0=dsv,
                                            in1=w4v[:, :, 160:WN], op=MU)
                    t3t = mp.tile([P, gs * MUL1], bf, tag="t3")
                    t3v = t3t[:].rearrange("p (t u) -> p t u", u=MUL1)
                    vtt(
                        out=t3v, in0=w4v[:, :, 128:160],
                        in1=at4[:, :, 0:1].broadcast_to([P, gs, MUL1]), op=MU)
                    vtt(
                        out=msg1[:, :, :, 0:MUL0].rearrange(
                            "p t c (v r) -> p t c v r", r=8),
                        in0=twv[:, :, 1:2, :].broadcast_to(
                            [P, gs, 3, MUL0]).rearrange(
                            "p t c (v r) -> p t c v r", r=8),
                        in1=a84.unsqueeze(3).broadcast_to([P, gs, 3, MUL0 // 8, 8]),
                        op=MU)
                    vtt(
                        out=msg1[:, :, :, MUL0:96],
                        in0=t3v.unsqueeze(2).broadcast_to([P, gs, 3, MUL1]),
                        in1=ys1v, op=MU)

                    for j in range(gs):
                        t = g0 + j
                        if cfg.ab_no_sel and t > 0:
                            continue
                        nc.tensor.matmul(
                            out=z_ps[:], lhsT=S4[:, j * P:(j + 1) * P],
                            rhs=msg4[:, j * D_MID:(j + 1) * D_MID],
                            start=(t == 0),
                            stop=(t == tt - 1) or cfg.ab_no_sel)

                # ---- node phase for this window
                if cfg.ab_no_node:
                    out_sb = mp.tile([P, D_IN], f32, tag="outsb")
                    nc.vector.tensor_copy(
                        out=out_sb[:], in_=s_store[:, w * D_IN:(w + 1) * D_IN])
                    nc.sync.dma_start(out=out_d[w * P:(w + 1) * P, :],
                                      in_=out_sb[:])
                    continue
                z_sb = mp.tile([P, D_MID], bf, tag="zsb")
                nc.scalar.activation(out=z_sb[:], in_=z_ps[:],
                                     func=mybir.ActivationFunctionType.Copy)
                o_ps = psN.tile([P, D_IN], f32, tag="o")
                for b in range(4):
                    zT_ps = psN.tile([96, P], bf, tag="o")
                    nc.tensor.transpose(out=zT_ps[:], in_=z_sb[:, b * 96:(b + 1) * 96],
                                        identity=ident[:])
                    zT_sb = mp.tile([96, P], bf, tag="zts")
                    nc.scalar.activation(out=zT_sb[:], in_=zT_ps[:],
                                         func=mybir.ActivationFunctionType.Copy)
                    if b == 0:
                        nc.tensor.matmul(out=o_ps[:, 0:MUL0], lhsT=zT_sb[:],
                                         rhs=Wl20_sb[:], start=True, stop=True)
                    else:
                        c = b - 1
                        nc.tensor.matmul(
                            out=o_ps[:, MUL0 + c * MUL1:MUL0 + (c + 1) * MUL1],
                            lhsT=zT_sb[:], rhs=Wl21_sb[:], start=True, stop=True)
                out_sb = mp.tile([P, D_IN], f32, tag="outsb")
                nc.vector.tensor_tensor(out=out_sb[:], in0=o_ps[:],
                                        in1=s_store[:, w * D_IN:(w + 1) * D_IN],
                                        op=AD)
                nc.sync.dma_start(out=out_d[w * P:(w + 1) * P, :], in_=out_sb[:])
            ctxN.__exit__(None, None, None)
            ctxZ.__exit__(None, None, None)
            ctxE.__exit__(None, None, None)

    nc.compile()
    _PROG_CACHE[cfg] = nc
    return nc


# ---------------------------------------------------------------- entry point

def _assemble(cfg: Cfg, results, node_attr):
    outs = [results[k]["out"][:cfg.npc] for k in range(cfg.n_cores)]
    o = np.concatenate(outs, axis=0).astype(F32)
    o = np.concatenate([o[:, :MUL0], _from_cmajor(o[:, MUL0:])], axis=1)
    return o * node_attr


def kernel(**inputs):
    cfg, in_maps, node_attr = _prep(inputs, n_cores=8)
    nc = _build(cfg)
    res = run_bass_kernel_spmd(nc, in_maps, core_ids=list(range(cfg.n_cores)))
    return _assemble(cfg, res.results, node_attr)

